# revision 40
# baseline (speedup 1.0000x reference)
"""AttentionBasedSampler Trainium2 kernel v2: 8-way token-sharded transformer.

Sharding: B=2 batches x 4-way token split -> 8 cores. Core c handles batch
c//4, token rows 192*(c%4) .. +192. Weights replicated per core.

v2 design (recipe validated in fp64 sim at ~9.5e-3 rel err, gate 2e-2):
  - fp8(e4m3) for qkv/sp/op/se paths (weights host-scaled x16), q/k/v/P/o/sb
    tiles fp8; ff1/ff2/out_w + their activations stay bf16; residual f32.
  - DoubleRow fp8 matmuls (K=256/instruction). Walrus requires the two
    k-tiles of each DR operand pair to be contiguous, so DR weights are
    host-reordered into [p, kpair, nblock, 2, n] layouts.
  - AllGather payload is xn^T fp8 (0.59MB/layer); k^T/v recomputed per core
    from two gathered copies (xnTfA/B for k^T rhs, xnTg for v lhsT).
  - sw^T folded into S via identity-matmul PSUM accumulate; fused
    exp((S+sw)/2048) ACT pass writes P directly as fp8.
  - Weight pools double-buffered (next layer streams during current).
Scales: S psum = 256*qk = 2048*logits -> exp scale 1/2048; swT stored as
2048*sw (ACT scale 128 from 16sw psum); op psum = 256*dx -> residual add
scales 1/256; LN scale-invariance cancels the SE h x16.
"""

import sys

sys.path.insert(0, "/opt/trn_rl_repo")

import numpy as np
import ml_dtypes

import concourse.bass as bass
import concourse.bacc as bacc
import concourse.mybir as mybir
import concourse.tile as tile
from concourse.masks import make_identity
from concourse.bass_utils import run_bass_kernel_spmd

F32 = mybir.dt.float32
BF16 = mybir.dt.bfloat16
F8 = mybir.dt.float8e4
AF = mybir.ActivationFunctionType
ALU = mybir.AluOpType
DRM = mybir.MatmulPerfMode.DoubleRow

B, L, D, H, FF, NL, HD = 2, 768, 768, 12, 2048, 6, 64
P = 128
DT = D // P            # 6 d-tiles
FFT = FF // P          # 16 ff-tiles
KT = D // P            # 6 key tiles (L == D == 768)
RWS = L // 4           # 192 rows per core
MTS = [(0, P), (P, RWS - P)]   # row M-tiles: (offset, size)
NT2 = [(0, 384), (384, 384)]
GROUPS = [[0, 1, 2, 3], [4, 5, 6, 7]]
EPS = 1e-5
WS = 16.0                      # host weight scale for fp8 paths
EXPS = 1.0 / (WS * WS * 8.0)   # exp scale
SWS = WS * 8.0                 # swT copy scale (16sw psum -> 2048sw)
OPS = 1.0 / (WS * WS)          # op residual descale

_CACHE = {}


def _bf(a):
    return np.ascontiguousarray(np.asarray(a, dtype=ml_dtypes.bfloat16))


def _f8(a, scale=1.0):
    return np.ascontiguousarray(
        np.asarray(np.asarray(a, np.float32) * scale,
                   dtype=ml_dtypes.float8_e4m3))


def _dr(w, nb):
    """[D', N] -> [p, dp, nblocks, 2, nb]: contiguous DR operand pairs."""
    Dw, N = w.shape
    return np.transpose(w.reshape(Dw // 256, 2, P, N // nb, nb),
                        (2, 0, 3, 1, 4))


def build_nc():
    nc = bacc.Bacc("TRN2", target_bir_lowering=False, debug=False, num_devices=8)

    # ---- I/O ----
    x_in = nc.dram_tensor("x_rows", [RWS, D], F32, kind="ExternalInput")
    si_in = nc.dram_tensor("siT8", [P, 3, 2 * RWS], F8, kind="ExternalInput")
    wse1_in = nc.dram_tensor("wse1", [P, 3, 4, 2, 512], F8, kind="ExternalInput")
    wse2_in = nc.dram_tensor("wse2", [P, 8, 6, 2, P], F8, kind="ExternalInput")
    wqk_in = nc.dram_tensor("wqk", [NL, P, 3, 12, 2, P], F8, kind="ExternalInput")
    wv_in = nc.dram_tensor("wv", [NL, P, 3, 2, 2, 384], F8, kind="ExternalInput")
    wsp_in = nc.dram_tensor("wsp", [NL, P, 3, 6, 2, P], F8, kind="ExternalInput")
    wop_in = nc.dram_tensor("wop", [NL, P, 3, 2, 2, 384], F8, kind="ExternalInput")
    wff1_in = nc.dram_tensor("wff1", [NL, D, FF], BF16, kind="ExternalInput")
    wff2_in = nc.dram_tensor("wff2", [NL, FF, D], BF16, kind="ExternalInput")
    wout_in = nc.dram_tensor("wout", [D, D], BF16, kind="ExternalInput")
    emat_in = nc.dram_tensor("emat", [2, P], F32, kind="ExternalInput")
    out_dram = nc.dram_tensor("out_rows", [RWS, D], F32, kind="ExternalOutput")

    with tile.TileContext(nc) as tc:
        with (
            tc.tile_pool(name="persist", bufs=1) as pp,
            tc.tile_pool(name="acts", bufs=1) as ap,
            tc.tile_pool(name="wts", bufs=2) as wp,
            tc.tile_pool(name="small", bufs=2) as sp,
            tc.tile_pool(name="ps", bufs=2, space="PSUM") as ps,
            tc.tile_pool(name="dram", bufs=2, space="DRAM") as dp,
        ):
            # ---- persistent tiles ----
            idf = pp.tile([P, P], F32)
            make_identity(nc, idf[:])
            i16 = pp.tile([P, P], BF16)
            nc.vector.tensor_copy(i16[:], idf[:])
            emat = pp.tile([2, P], F32)
            nc.sync.dma_start(emat[:], emat_in[:])

            F32R_ = mybir.dt.float32r
            i1f = pp.tile([P, P], F32R_)
            nc.vector.tensor_copy(i1f[:], idf[:])
            i256f = pp.tile([P, P], F32R_)
            nc.vector.tensor_scalar(out=i256f[:], in0=idf[:], scalar1=256.0,
                                    scalar2=None, op0=ALU.mult)
            x_sb = pp.tile([P, 2, D], F32R_)
            x_stage = ap.tile([P, 2, D], F32, tag="hse")
            nc.sync.dma_start(x_stage[:, 0, :], x_in[0:P, :])
            nc.sync.dma_start(x_stage[0:RWS - P, 1, :], x_in[P:RWS, :])
            for mt, (mo, mp_) in enumerate(MTS):
                nc.vector.tensor_copy(x_sb[0:mp_, mt, :], x_stage[0:mp_, mt, :])
            sbT8 = pp.tile([P, DT, RWS], F8)    # SE output ^T (fp8)

            def lhsT_blk(tile_, dpi, mt):
                """DR-lhsT slice for row-block mt of a [P, 3, 384] tile."""
                if mt == 0:
                    return tile_[:, dpi, 0:256].rearrange(
                        "p (j m) -> p j m", j=2)
                return tile_[:, dpi, 256:384].rearrange(
                    "p (j m) -> p j m", j=2)

            # ---------- layer-norm helper (bn_stats + ACT rsqrt) ----------
            def layernorm(src_tile, width, out_t, func=AF.Identity):
                nch = (width + 511) // 512
                st6 = sp.tile([P, 2, nch, 6], F32, tag=f"lnst6_{width}")
                mv = sp.tile([P, 2, 2], F32, tag="lnmv")
                rs = sp.tile([P, 2], F32, tag="lnrs")
                nmrs = sp.tile([P, 2], F32, tag="lnnm")
                cw = width // nch
                for mt, (mo, mp_) in enumerate(MTS):
                    for c in range(nch):
                        nc.vector.bn_stats(
                            st6[0:mp_, mt, c],
                            src_tile[0:mp_, mt, cw * c:cw * (c + 1)])
                    nc.vector.bn_aggr(mv[0:mp_, mt], st6[0:mp_, mt])
                vr = sp.tile([P, 2], F32, tag="lnvr")
                nc.vector.tensor_scalar(out=vr[:], in0=mv[:, :, 1],
                                        scalar1=EPS, scalar2=None, op0=ALU.add)
                nc.vector.reciprocal(rs[:], vr[:])
                nc.scalar.activation(rs[:], rs[:], AF.Sqrt)
                nc.vector.scalar_tensor_tensor(
                    out=nmrs[:], in0=mv[:, :, 0], scalar=-1.0, in1=rs[:],
                    op0=ALU.mult, op1=ALU.mult)
                for mt, (mo, mp_) in enumerate(MTS):
                    nc.scalar.activation(
                        out_t[0:mp_, mt, :], src_tile[0:mp_, mt, :],
                        func,
                        bias=nmrs[0:mp_, mt:mt + 1], scale=rs[0:mp_, mt:mt + 1],
                    )

            F32R = mybir.dt.float32r

            def ln_boundary(emit_nh, ident, xscale, out_t, func=AF.Identity,
                            tag=""):
                """Residual+LN fused at a layer boundary.

                emit_nh(psum, mt, mp_, nh): emits the delta matmuls
                (start=False accumulation) into psum[0:mp_, :].
                psum is pre-loaded with ident.T @ x (so psum = s*(x+delta));
                LN stats+apply read the psums; x_sb updated off-chain with
                xscale."""
                st6b = sp.tile([P, 2, 2, 6], F32, tag="lnst6_768")
                mvb = sp.tile([P, 2, 2], F32, tag="lnmv")
                rsb = sp.tile([P, 2], F32, tag="lnrs")
                nmb = sp.tile([P, 2], F32, tag="lnnm")
                vrb = sp.tile([P, 2], F32, tag="lnvr")
                for mt, (mo, mp_) in enumerate(MTS):
                    pss = []
                    for nh in range(2):
                        pb = ps.tile([P, 384], F32, tag="t384", bufs=5,
                                     name=f"rb{tag}_{mt}_{nh}")
                        nc.tensor.matmul(
                            pb[0:mp_, :], ident[0:mp_, 0:mp_],
                            x_sb[0:mp_, mt, 384 * nh:384 * (nh + 1)],
                            start=True, stop=False)
                        emit_nh(pb, mt, mp_, nh)
                        pss.append(pb)
                        nc.vector.bn_stats(st6b[0:mp_, mt, nh], pb[0:mp_, :])
                    nc.vector.bn_aggr(mvb[0:mp_, mt], st6b[0:mp_, mt])
                    nc.vector.tensor_scalar(
                        out=vrb[0:mp_, mt:mt + 1], in0=mvb[0:mp_, mt, 1:2],
                        scalar1=EPS, scalar2=None, op0=ALU.add)
                    nc.vector.reciprocal(rsb[0:mp_, mt:mt + 1],
                                         vrb[0:mp_, mt:mt + 1])
                    nc.scalar.activation(rsb[0:mp_, mt:mt + 1],
                                         rsb[0:mp_, mt:mt + 1], AF.Sqrt)
                    nc.vector.scalar_tensor_tensor(
                        out=nmb[0:mp_, mt:mt + 1], in0=mvb[0:mp_, mt, 0:1],
                        scalar=-1.0, in1=rsb[0:mp_, mt:mt + 1],
                        op0=ALU.mult, op1=ALU.mult)
                    for nh in range(2):
                        nc.scalar.activation(
                            out_t[0:mp_, mt, 384 * nh:384 * (nh + 1)],
                            pss[nh][0:mp_, :], func,
                            bias=nmb[0:mp_, mt:mt + 1],
                            scale=rsb[0:mp_, mt:mt + 1])
                        nc.scalar.activation(
                            x_sb[0:mp_, mt, 384 * nh:384 * (nh + 1)],
                            pss[nh][0:mp_, :], AF.Identity, scale=xscale)

            # ---------- transpose helper (bf16 through PE) ----------
            def transpose_rows(src, out_T, dtiles):
                for d0 in range(0, dtiles, DT):
                    nd = min(DT, dtiles - d0)
                    for mt, (mo, mp_) in enumerate(MTS):
                        pt = ps.tile([P, DT, P], BF16, tag="t384", bufs=5)
                        for dd in range(nd):
                            nc.tensor.transpose(
                                pt[0:P, dd, 0:mp_],
                                src[0:mp_, mt, P * (d0 + dd):P * (d0 + dd + 1)],
                                i16[0:mp_, 0:mp_],
                            )
                        nc.vector.tensor_copy(
                            out_T[:, d0:d0 + nd, mo:mo + mp_],
                            pt[0:P, 0:nd, 0:mp_])

            # ---- L0 xn + AllGather issued first: overlaps the SE ----
            xn_cur = ap.tile([P, 2, D], BF16, tag="xn", name="xn0")
            layernorm(x_stage, D, xn_cur)

            def issue_ag(xnT8):
                ag_in = dp.tile([P, DT * RWS], F8, tag="agin")
                nc.sync.dma_start(ag_in[:],
                                  xnT8[:].rearrange("p t r -> p (t r)"))
                ag_out = dp.tile([4, P, DT * RWS], F8, tag="agout")
                nc.gpsimd.collective_compute(
                    "AllGather", ALU.bypass, replica_groups=GROUPS,
                    ins=[ag_in.opt()], outs=[ag_out.opt()],
                )
                return ag_out

            xnT8_cur = ap.tile([P, DT, RWS], F8, tag="xnTown", name="xnT0")
            transpose_rows(xn_cur, xnT8_cur, DT)
            ag_out_cur = issue_ag(xnT8_cur)

            # ================= structure encoder (fp8 DR) =================
            siT8 = pp.tile([P, 3, 2 * RWS], F8)
            nc.gpsimd.dma_start(siT8[:], si_in[:])
            wse1 = wp.tile([P, 3, 4, 2, 512], F8, tag="wff1")
            nc.gpsimd.dma_start(wse1[:], wse1_in[:])
            h_sb = ap.tile([P, 2, FF], BF16, tag="hse")
            for mt, (mo, mp_) in enumerate(MTS):
                for nb in range(4):
                    hp = ps.tile([P, 512], F32, tag="t384", bufs=5)
                    for dpi in range(3):
                        nc.tensor.matmul(
                            hp[0:mp_, :], lhsT_blk(siT8, dpi, mt),
                            wse1[:, dpi, nb, :, :],
                            start=(dpi == 0), stop=(dpi == 2), perf_mode=DRM,
                        )
                    nc.vector.tensor_copy(
                        h_sb[0:mp_, mt, 512 * nb:512 * (nb + 1)], hp[0:mp_, :])
            hr = ap.tile([P, 2, FF], BF16, tag="hrse")
            layernorm(h_sb, FF, hr, func=AF.Relu)
            hrT8 = ap.tile([P, FFT, RWS], F8, tag="hrT")
            transpose_rows(hr, hrT8, FFT)
            wse2 = wp.tile([P, 8, 6, 2, P], F8, tag="wff2", bufs=1)
            nc.gpsimd.dma_start(wse2[:], wse2_in[:])
            for m in range(DT):
                sbp = ps.tile([P, RWS], F32, tag="t192o", bufs=3)
                for fp_ in range(8):
                    nc.tensor.matmul(
                        sbp[:], wse2[:, fp_, m, :, :],
                        hrT8[:, 2 * fp_:2 * fp_ + 2, :],
                        start=(fp_ == 0), stop=(fp_ == 7), perf_mode=DRM,
                    )
                nc.scalar.activation(sbT8[:, m, :], sbp[:], AF.Identity,
                                     scale=1.0 / WS)

            # ---- sw^T for layer li: swT bf16 = 2048*sw ----
            def emit_swT(li):
                wsp = wp.tile([P, 3, 6, 2, P], F8, tag="wsp", name=f"wsp{li}")
                nc.gpsimd.dma_start(wsp[:], wsp_in[li])
                swT = ap.tile([P, KT, RWS], BF16, tag="swT", bufs=2,
                              name=f"swT{li}")
                for ktp in range(3):
                    swp = ps.tile([P, 2, RWS], F32, tag="t384", bufs=5)
                    for j2 in range(2):
                        kt = 2 * ktp + j2
                        for dpi in range(3):
                            nc.tensor.matmul(
                                swp[:, j2, :], wsp[:, dpi, kt, :, :],
                                sbT8[:, 2 * dpi:2 * dpi + 2, :],
                                start=(dpi == 0), stop=(dpi == 2),
                                perf_mode=DRM,
                            )
                    nc.scalar.activation(
                        swT[:, 2 * ktp:2 * ktp + 2, :].rearrange(
                            "p a b -> p (a b)"),
                        swp[:].rearrange("p a b -> p (a b)"),
                        AF.Identity, scale=SWS)
                return swT

            swT_cur = emit_swT(0)

            v8 = pp.tile([P, 3, H, 2, P], F8)
            nc.gpsimd.memset(v8[:, :, :, :, HD + 1:P], 0.0)
            nc.gpsimd.memset(v8[:, :, :, :, HD:HD + 1], 1.0)

            # ================= transformer layers =================
            for li in range(NL):
                swT = swT_cur
                wqk = wp.tile([P, 3, 12, 2, P], F8, tag="wqk")
                nc.gpsimd.dma_start(wqk[:], wqk_in[li])
                wv = wp.tile([P, 3, 2, 2, 384], F8, tag="wv")
                nc.gpsimd.dma_start(wv[:], wv_in[li])
                wopt = wp.tile([P, 3, 2, 2, 384], F8, tag="wop")
                nc.gpsimd.dma_start(wopt[:], wop_in[li])
                wff1 = wp.tile([P, DT, FF], BF16, tag="wff1")
                nc.gpsimd.dma_start(
                    wff1[:], wff1_in[li].rearrange("(t p) n -> p t n", p=P))
                wff2 = wp.tile([P, FFT, D], BF16, tag="wff2", bufs=1)
                nc.gpsimd.dma_start(
                    wff2[:], wff2_in[li].rearrange("(t p) n -> p t n", p=P))

                # ---- AG for this layer was issued at the prev boundary ----
                xnT8 = xnT8_cur
                ag_out = ag_out_cur

                # ---- AG-window work: q^T + next layer's swT ----
                qT8 = ap.tile([P, DT, RWS], F8, tag="qT")
                for fp_ in range(3):
                    qp = ps.tile([P, 2, RWS], F32, tag="t384", bufs=5)
                    for j2 in range(2):
                        f = 2 * fp_ + j2
                        for dpi in range(3):
                            nc.tensor.matmul(
                                qp[:, j2, :], wqk[:, dpi, f, :, :],
                                xnT8[:, 2 * dpi:2 * dpi + 2, :],
                                start=(dpi == 0), stop=(dpi == 2),
                                perf_mode=DRM,
                            )
                    nc.vector.tensor_copy(
                        qT8[:, 2 * fp_:2 * fp_ + 2, :].rearrange(
                            "p a b -> p (a b)"),
                        qp[:].rearrange("p a b -> p (a b)"))
                if li + 1 < NL:
                    swT_cur = emit_swT(li + 1)
                # ---- distribute gathered xn^T ----
                xnTg = ap.tile([P, KT, 3, 2, P], F8, tag="xnTg")
                dmaq = [nc.sync, nc.gpsimd]
                qi = 0
                for g in range(4):
                    src = ag_out[g].rearrange("p (a j r) -> p a j r", a=3, j=2)
                    key0 = RWS * g
                    segs = ([(0, P), (P, 64)] if g % 2 == 0
                            else [(0, 64), (64, P)])
                    for r0, ln_ in segs:
                        key = key0 + r0
                        dmaq[qi % 2].dma_start(
                            xnTg[:, key // P, :, :, key % P:key % P + ln_],
                            src[:, :, :, r0:r0 + ln_])
                        qi += 1

                # ---- k^T / v full (DR), interleaved so attention can
                #      start after the first tiles land ----
                kT8 = ap.tile([P, DT, L], F8, tag="kT8")

                def emit_kT(f, nh):
                    kp = ps.tile([P, 384], F32, tag="t384", bufs=5)
                    for c in range(3):
                        kt = 3 * nh + c
                        for dpi in range(3):
                            nc.tensor.matmul(
                                kp[:, P * c:P * (c + 1)],
                                wqk[:, dpi, 6 + f, :, :],
                                xnTg[:, kt, dpi, :, :],
                                start=(dpi == 0), stop=(dpi == 2),
                                perf_mode=DRM,
                            )
                    if (2 * f + nh) % 2 == 0:
                        nc.scalar.activation(
                            kT8[:, f, 384 * nh:384 * (nh + 1)], kp[:],
                            AF.Identity)
                    else:
                        nc.vector.tensor_copy(
                            kT8[:, f, 384 * nh:384 * (nh + 1)], kp[:])

                def emit_v(kt, nh):
                    vp = ps.tile([P, 384], F32, tag="t384", bufs=5)
                    for dpi in range(3):
                        nc.tensor.matmul(
                            vp[:], xnTg[:, kt, dpi, :, :],
                            wv[:, dpi, nh, :, :],
                            start=(dpi == 0), stop=(dpi == 2),
                            perf_mode=DRM,
                        )
                    dst = v8[:, kt // 2, 6 * nh:6 * nh + 6, kt % 2, 0:HD]
                    src_v = vp[:].rearrange("p (h d) -> p h d", d=HD)
                    if (2 * kt + nh) % 2 == 0:
                        nc.vector.tensor_copy(dst, src_v)
                    else:
                        nc.scalar.activation(dst, src_v, AF.Identity)

                for i in range(DT):
                    emit_kT(i, 0)
                    emit_kT(i, 1)
                for i in range(DT):
                    emit_v(i, 0)
                    emit_v(i, 1)

                # ---- attention ----
                oT8 = ap.tile([P, 3, 384], F8, tag="oT")   # DR-lhsT layout
                for hpair in range(DT):
                    opair = [ps.tile([P, RWS], F32, tag="t192o", bufs=3,
                                     name=f"op{li}_{hpair}_{_h}")
                             for _h in range(2)]
                    pexps_all = []
                    for ktp in range(3):
                        spair = [ps.tile([P, 2, RWS], F32, tag="t384", bufs=5,
                                         name=f"sp{li}_{hpair}_{ktp}_{_h}")
                                 for _h in range(2)]
                        for j in range(2):
                            kt = 2 * ktp + j
                            for hh in range(2):
                                po = 64 * hh
                                nc.tensor.matmul(
                                    spair[hh][:, j, :], i16[:],
                                    swT[:, kt, :],
                                    start=True, stop=False,
                                )
                                nc.tensor.matmul(
                                    spair[hh][:, j, :],
                                    kT8[po:po + HD, hpair, P * kt:P * (kt + 1)],
                                    qT8[po:po + HD, hpair, :],
                                    start=False, stop=True,
                                )
                        pexps = []
                        for hh in range(2):
                            pexp = sp.tile([P, 2, RWS], F8, tag="pexp", bufs=8,
                                           name=f"px{li}_{hpair}_{ktp}_{hh}")
                            pexps.append(pexp)
                            nc.scalar.activation(
                                pexp[:].rearrange("p a b -> p (a b)"),
                                spair[hh][:].rearrange("p a b -> p (a b)"),
                                AF.Exp, scale=EXPS)
                        pexps_all.append(pexps)
                    for ktp in range(3):
                        for hh in range(2):
                            h = 2 * hpair + hh
                            nc.tensor.matmul(
                                opair[hh][:],
                                v8[:, ktp, h, :, :],
                                pexps_all[ktp][hh][:],
                                start=(ktp == 0), stop=(ktp == 2),
                                perf_mode=DRM,
                            )
                    rz = sp.tile([1, 2, RWS], F32, tag="rz", bufs=1)
                    for hh in range(2):
                        nc.vector.reciprocal(rz[0:1, hh, :],
                                             opair[hh][HD:HD + 1, :])
                    rbp = ps.tile([P, RWS], F32, tag="t192o", bufs=3)
                    for hh in range(2):
                        nc.tensor.matmul(rbp[64 * hh:64 * hh + 64, :],
                                         emat[0:1, 0:64], rz[0:1, hh, :],
                                         start=True, stop=True)
                    rb = sp.tile([P, RWS], F32, tag="rb")
                    nc.vector.tensor_copy(rb[:], rbp[:])
                    dpi, jj = divmod(hpair, 2)
                    for hh in range(2):
                        po = 64 * hh
                        nc.vector.tensor_tensor(
                            out=oT8[po:po + HD, dpi, P * jj:P * jj + P],
                            in0=opair[hh][0:HD, 0:P], in1=rb[po:po + HD, 0:P],
                            op=ALU.mult)
                        nc.vector.tensor_tensor(
                            out=oT8[po:po + HD, dpi,
                                    256 + 64 * jj:320 + 64 * jj],
                            in0=opair[hh][0:HD, P:RWS],
                            in1=rb[po:po + HD, P:RWS],
                            op=ALU.mult)

                # ---- output projection + residual + LN2 (psum-fused) ----
                def emit_op(pb, mt, mp_, nh):
                    for dpi in range(3):
                        nc.tensor.matmul(
                            pb[0:mp_, :], lhsT_blk(oT8, dpi, mt),
                            wopt[:, dpi, nh, :, :],
                            start=False, stop=(dpi == 2),
                            perf_mode=DRM,
                        )
                xn2 = ap.tile([P, 2, D], BF16, tag="xn", name=f"xn2_{li}")
                ln_boundary(emit_op, i256f, OPS, xn2, tag=f"o{li}")
                xn2T = ap.tile([P, DT, RWS], BF16, tag="xn2T")
                transpose_rows(xn2, xn2T, DT)
                h1T = ap.tile([P, FFT, RWS], BF16, tag="h1T")
                for fp_ in range(0, FFT, 2):
                    fps_ = ps.tile([P, 2, RWS], F32, tag="t384", bufs=5)
                    for j in range(2):
                        f = fp_ + j
                        for d in range(DT):
                            nc.tensor.matmul(
                                fps_[:, j, :], wff1[:, d, P * f:P * (f + 1)],
                                xn2T[:, d, :],
                                start=(d == 0), stop=(d == DT - 1),
                            )
                    nc.scalar.activation(
                        h1T[:, fp_:fp_ + 2, :].rearrange("p a b -> p (a b)"),
                        fps_[:].rearrange("p a b -> p (a b)"), AF.Relu)
                def emit_ff2(pb, mt, mp_, nh):
                    mo = MTS[mt][0]
                    for f in range(FFT):
                        nc.tensor.matmul(
                            pb[0:mp_, :], h1T[:, f, mo:mo + mp_],
                            wff2[:, f, 384 * nh:384 * (nh + 1)],
                            start=False, stop=(f == FFT - 1),
                        )
                if li + 1 < NL:
                    xn_cur = ap.tile([P, 2, D], BF16, tag="xn",
                                     name=f"xn{li + 1}")
                    ln_boundary(emit_ff2, i1f, 1.0, xn_cur, tag=f"f{li}")
                    xnT8_cur = ap.tile([P, DT, RWS], F8, tag="xnTown",
                                       name=f"xnT{li + 1}")
                    transpose_rows(xn_cur, xnT8_cur, DT)
                    ag_out_cur = issue_ag(xnT8_cur)
                else:
                    xf = ap.tile([P, 2, D], BF16, tag="xn", name="xf")
                    for mt, (mo, mp_) in enumerate(MTS):
                        for nh, (n0, nw) in enumerate(NT2):
                            f2p = ps.tile([P, 384], F32, tag="t384", bufs=5)
                            nc.tensor.matmul(
                                f2p[0:mp_, :], i1f[0:mp_, 0:mp_],
                                x_sb[0:mp_, mt, n0:n0 + nw],
                                start=True, stop=False)
                            for f in range(FFT):
                                nc.tensor.matmul(
                                    f2p[0:mp_, 0:nw], h1T[:, f, mo:mo + mp_],
                                    wff2[:, f, n0:n0 + nw],
                                    start=False, stop=(f == FFT - 1),
                                )
                            nc.scalar.activation(
                                xf[0:mp_, mt, n0:n0 + nw], f2p[0:mp_, :],
                                AF.Identity)

            # ================= final projection (bf16) =================
            xfT = ap.tile([P, DT, RWS], BF16, tag="xn2T")
            transpose_rows(xf, xfT, DT)
            wout = wp.tile([P, DT, D], BF16, tag="wout", bufs=1)
            nc.gpsimd.dma_start(wout[:], wout_in.rearrange("(t p) n -> p t n", p=P))
            out_sb = pp.tile([P, 2, D], F32)
            for mt, (mo, mp_) in enumerate(MTS):
                for n0, nw in NT2:
                    fop = ps.tile([P, 384], F32, tag="t384", bufs=5)
                    for d in range(DT):
                        nc.tensor.matmul(
                            fop[0:mp_, 0:nw], xfT[:, d, mo:mo + mp_],
                            wout[:, d, n0:n0 + nw],
                            start=(d == 0), stop=(d == DT - 1),
                        )
                    nc.vector.tensor_copy(out_sb[0:mp_, mt, n0:n0 + nw],
                                          fop[0:mp_, 0:nw])
                    nc.sync.dma_start(out_dram[mo:mo + mp_, n0:n0 + nw],
                                      out_sb[0:mp_, mt, n0:n0 + nw])

    nc.finalize()
    return nc


def _si_dr(siT):
    """siT [D, RWS] -> [P, 3, 384] DR-lhsT with row blocks (128, 64)."""
    A = siT.reshape(3, 2, P, RWS).transpose(2, 0, 1, 3)   # [p, dp, j, r]
    blk0 = np.ascontiguousarray(A[:, :, :, 0:P]).reshape(P, 3, 256)
    blk1 = np.ascontiguousarray(A[:, :, :, P:RWS]).reshape(P, 3, P)
    return np.concatenate([blk0, blk1], axis=-1)


def kernel(**inputs):
    inp = {k: np.asarray(v, dtype=np.float32) for k, v in inputs.items()}

    qkv_w = inp["qkv_w"].copy()
    qkv_b = inp["qkv_b"].copy()
    for i in range(NL):
        g, b = inp["n1_g"][i], inp["n1_b"][i]
        qkv_b[i] = qkv_b[i] + b @ qkv_w[i]
        qkv_w[i] = g[:, None] * qkv_w[i]
    ff_w1 = inp["ff_w1"].copy()
    ff_b1 = inp["ff_b1"].copy()
    for i in range(NL):
        g, b = inp["n2_g"][i], inp["n2_b"][i]
        ff_b1[i] = ff_b1[i] + b @ ff_w1[i]
        ff_w1[i] = g[:, None] * ff_w1[i]
    sp_b = inp["sp_b"] + inp["se_b2"] @ inp["sp_w"]

    unsupported = []
    for name, arr in [("qkv_b", qkv_b), ("sp_b", sp_b), ("op_b", inp["op_b"]),
                      ("ff_b1", ff_b1), ("ff_b2", inp["ff_b2"]),
                      ("se_b1", inp["se_b1"]), ("out_b", inp["out_b"])]:
        if np.abs(arr).max() > 0:
            unsupported.append(name)
    if (inp["se_g"] != 1).any() or (inp["se_be"] != 0).any():
        unsupported.append("se_affine")
    assert not unsupported, f"nonzero biases not yet supported: {unsupported}"

    wqk = np.stack([_dr(qkv_w[i][:, 0:2 * D], P) for i in range(NL)])
    wv = np.stack([_dr(qkv_w[i][:, 2 * D:3 * D], 384) for i in range(NL)])
    wsp = np.stack([_dr(inp["sp_w"][i], P) for i in range(NL)])
    wop = np.stack([_dr(inp["op_w"][i], 384) for i in range(NL)])
    wse1 = _dr(inp["se_w1"], 512)
    wse2 = _dr(inp["se_w2"], P)

    emat_np = np.zeros((2, P), dtype=np.float32)
    emat_np[0, 0:HD] = 1.0
    emat_np[1, HD:2 * HD] = 1.0
    if "nc" not in _CACHE:
        _CACHE["nc"] = build_nc()
    nc = _CACHE["nc"]

    in_maps = []
    for c in range(8):
        b, j = divmod(c, 4)
        rows = slice(RWS * j, RWS * (j + 1))
        in_maps.append({
            "x_rows": np.ascontiguousarray(inp["x"][b, rows]),
            "siT8": _f8(_si_dr(np.ascontiguousarray(
                inp["structure_info"][b, rows].T))),
            "wse1": _f8(wse1, WS), "wse2": _f8(wse2, WS),
            "wqk": _f8(wqk, WS), "wv": _f8(wv, WS),
            "wsp": _f8(wsp, WS), "wop": _f8(wop, WS),
            "wff1": _bf(ff_w1), "wff2": _bf(inp["ff_w2"]),
            "wout": _bf(inp["out_w"]), "emat": emat_np,
        })

    res = run_bass_kernel_spmd(nc, in_maps, core_ids=list(range(8)),
                               **_CACHE.get("run_kwargs", {}))
    _CACHE["last_result"] = res
    out = np.zeros((B, L, D), dtype=np.float32)
    for c in range(8):
        b, j = divmod(c, 4)
        out[b, RWS * j:RWS * (j + 1)] = res.results[c]["out_rows"]
    return out


if __name__ == "__main__":
    import reference as R
    import os
    os.environ["JAX_PLATFORMS"] = "cpu"
    inputs = {k: np.asarray(v) for k, v in R.setup_inputs().items()}
    got = kernel(**inputs)
    import jax.numpy as jnp
    want = np.asarray(R.reference(**{k: jnp.asarray(v) for k, v in inputs.items()}))
    err = np.abs(got - want).max() / np.abs(want).max()
    print("rel err:", err)



# revision 41
# speedup vs baseline: 1.3612x; 1.3612x over previous
"""AttentionBasedSampler Trainium2 kernel v2: 8-way token-sharded transformer.

Sharding: B=2 batches x 4-way token split -> 8 cores. Core c handles batch
c//4, token rows 192*(c%4) .. +192. Weights replicated per core.

v2 design (recipe validated in fp64 sim at ~9.5e-3 rel err, gate 2e-2):
  - fp8(e4m3) for qkv/sp/op/se paths (weights host-scaled x16), q/k/v/P/o/sb
    tiles fp8; ff1/ff2/out_w + their activations stay bf16; residual f32.
  - DoubleRow fp8 matmuls (K=256/instruction). Walrus requires the two
    k-tiles of each DR operand pair to be contiguous, so DR weights are
    host-reordered into [p, kpair, nblock, 2, n] layouts.
  - AllGather payload is xn^T fp8 (0.59MB/layer); k^T/v recomputed per core
    from two gathered copies (xnTfA/B for k^T rhs, xnTg for v lhsT).
  - sw^T folded into S via identity-matmul PSUM accumulate; fused
    exp((S+sw)/2048) ACT pass writes P directly as fp8.
  - Weight pools double-buffered (next layer streams during current).
Scales: S psum = 256*qk = 2048*logits -> exp scale 1/2048; swT stored as
2048*sw (ACT scale 128 from 16sw psum); op psum = 256*dx -> residual add
scales 1/256; LN scale-invariance cancels the SE h x16.
"""

import sys

sys.path.insert(0, "/opt/trn_rl_repo")

import numpy as np
import ml_dtypes

import concourse.bass as bass
import concourse.bacc as bacc
import concourse.mybir as mybir
import concourse.tile as tile
from concourse.masks import make_identity
from concourse.bass_utils import run_bass_kernel_spmd
import bass_rust

_add_dep = bass_rust.add_dep_helper

F32 = mybir.dt.float32
BF16 = mybir.dt.bfloat16
F16 = mybir.dt.float16
F8 = mybir.dt.float8e4
AF = mybir.ActivationFunctionType
ALU = mybir.AluOpType
DRM = mybir.MatmulPerfMode.DoubleRow

B, L, D, H, FF, NL, HD = 2, 768, 768, 12, 2048, 6, 64
P = 128
DT = D // P            # 6 d-tiles
FFT = FF // P          # 16 ff-tiles
KT = D // P            # 6 key tiles (L == D == 768)
RWS = L // 4           # 192 rows per core
MTS = [(0, P), (P, RWS - P)]   # row M-tiles: (offset, size)
NT2 = [(0, 384), (384, 384)]
GROUPS = [[0, 1, 2, 3], [4, 5, 6, 7]]
EPS = 1e-5
WS = 16.0                      # host weight scale for fp8 paths
EXPS = 1.0 / (WS * WS * 8.0)   # exp scale
SWS = WS * 8.0                 # swT copy scale (16sw psum -> 2048sw)
OPS = 1.0 / (WS * WS)          # op residual descale

_CACHE = {}


def _bf(a):
    return np.ascontiguousarray(np.asarray(a, dtype=ml_dtypes.bfloat16))


def _f8(a, scale=1.0):
    return np.ascontiguousarray(
        np.asarray(np.asarray(a, np.float32) * scale,
                   dtype=ml_dtypes.float8_e4m3))


def _dr(w, nb):
    """[D', N] -> [p, dp, nblocks, 2, nb]: contiguous DR operand pairs."""
    Dw, N = w.shape
    return np.transpose(w.reshape(Dw // 256, 2, P, N // nb, nb),
                        (2, 0, 3, 1, 4))


def build_nc():
    """Returns (nc, patches). patches = [(inner_inst, hw_wait_value)] for
    waits on remote-dma semaphores: emitted as >=0 (single-core sims can't
    model cross-core sem arrival), patched to the real thresholds for the
    HW run only."""
    nc = bacc.Bacc("TRN2", target_bir_lowering=False, debug=False, num_devices=8)

    # ---- I/O ----
    x_in = nc.dram_tensor("x_rows", [RWS, D], F32, kind="ExternalInput")
    si_in = nc.dram_tensor("siT8", [P, 3, 2 * RWS], F8, kind="ExternalInput")
    wse1_in = nc.dram_tensor("wse1", [P, 3, 4, 2, 512], F8, kind="ExternalInput")
    wse2_in = nc.dram_tensor("wse2", [P, 8, 6, 2, P], F8, kind="ExternalInput")
    wqk_in = nc.dram_tensor("wqk", [NL, P, 3, 12, 2, P], F8, kind="ExternalInput")
    wv_in = nc.dram_tensor("wv", [NL, P, 3, 2, 2, 384], F8, kind="ExternalInput")
    wsp_in = nc.dram_tensor("wsp", [NL, P, 3, 6, 2, P], F8, kind="ExternalInput")
    wop_in = nc.dram_tensor("wop", [NL, P, 3, 2, 2, 384], F8, kind="ExternalInput")
    wff1_in = nc.dram_tensor("wff1", [NL, D, FF], BF16, kind="ExternalInput")
    wff2_in = nc.dram_tensor("wff2", [NL, FF, D], BF16, kind="ExternalInput")
    wout_in = nc.dram_tensor("wout", [D, D], BF16, kind="ExternalInput")
    emat_in = nc.dram_tensor("emat", [2, P], F16, kind="ExternalInput")
    out_dram = nc.dram_tensor("out_rows", [RWS, D], F32, kind="ExternalOutput")

    with tile.TileContext(nc) as tc:
        with (
            tc.tile_pool(name="persist", bufs=1) as pp,
            tc.tile_pool(name="acts", bufs=1) as ap,
            tc.tile_pool(name="wts", bufs=2) as wp,
            tc.tile_pool(name="small", bufs=2) as sp,
            tc.tile_pool(name="ps", bufs=2, space="PSUM") as ps,
        ):
            # ---- persistent tiles ----
            idf = pp.tile([P, P], F32)
            make_identity(nc, idf[:])
            i16 = pp.tile([P, P], BF16)
            nc.vector.tensor_copy(i16[:], idf[:])
            emat = pp.tile([2, P], F16)
            nc.sync.dma_start(emat[:], emat_in[:])

            F32R_ = mybir.dt.float32r
            i1f = pp.tile([P, P], F32R_)
            nc.vector.tensor_copy(i1f[:], idf[:])
            i256f = pp.tile([P, P], F32R_)
            nc.vector.tensor_scalar(out=i256f[:], in0=idf[:], scalar1=256.0,
                                    scalar2=None, op0=ALU.mult)
            x_sb = pp.tile([P, 2, D], F32R_)
            x_stage = ap.tile([P, 2, D], F32, tag="hse")
            nc.sync.dma_start(x_stage[:, 0, :], x_in[0:P, :])
            nc.sync.dma_start(x_stage[0:RWS - P, 1, :], x_in[P:RWS, :])
            for mt, (mo, mp_) in enumerate(MTS):
                nc.vector.tensor_copy(x_sb[0:mp_, mt, :], x_stage[0:mp_, mt, :])
            sbT8 = pp.tile([P, DT, RWS], F8)    # SE output ^T (fp8)

            def lhsT_blk(tile_, dpi, mt):
                """DR-lhsT slice for row-block mt of a [P, 3, 384] tile."""
                if mt == 0:
                    return tile_[:, dpi, 0:256].rearrange(
                        "p (j m) -> p j m", j=2)
                return tile_[:, dpi, 256:384].rearrange(
                    "p (j m) -> p j m", j=2)

            # ---------- layer-norm helper (bn_stats + ACT rsqrt) ----------
            def layernorm(src_tile, width, out_t, func=AF.Identity):
                nch = (width + 511) // 512
                st6 = sp.tile([P, 2, nch, 6], F32, tag=f"lnst6_{width}")
                mv = sp.tile([P, 2, 2], F32, tag="lnmv")
                rs = sp.tile([P, 2], F32, tag="lnrs")
                nmrs = sp.tile([P, 2], F32, tag="lnnm")
                cw = width // nch
                for mt, (mo, mp_) in enumerate(MTS):
                    for c in range(nch):
                        nc.vector.bn_stats(
                            st6[0:mp_, mt, c],
                            src_tile[0:mp_, mt, cw * c:cw * (c + 1)])
                    nc.vector.bn_aggr(mv[0:mp_, mt], st6[0:mp_, mt])
                vr = sp.tile([P, 2], F32, tag="lnvr")
                nc.vector.tensor_scalar(out=vr[:], in0=mv[:, :, 1],
                                        scalar1=EPS, scalar2=None, op0=ALU.add)
                nc.vector.reciprocal(rs[:], vr[:])
                nc.scalar.activation(rs[:], rs[:], AF.Sqrt)
                nc.vector.scalar_tensor_tensor(
                    out=nmrs[:], in0=mv[:, :, 0], scalar=-1.0, in1=rs[:],
                    op0=ALU.mult, op1=ALU.mult)
                for mt, (mo, mp_) in enumerate(MTS):
                    nc.scalar.activation(
                        out_t[0:mp_, mt, :], src_tile[0:mp_, mt, :],
                        func,
                        bias=nmrs[0:mp_, mt:mt + 1], scale=rs[0:mp_, mt:mt + 1],
                    )

            F32R = mybir.dt.float32r

            def ln_boundary(emit_nh, ident, xscale, out_t, func=AF.Identity,
                            tag=""):
                """Residual+LN fused at a layer boundary.

                emit_nh(psum, mt, mp_, nh): emits the delta matmuls
                (start=False accumulation) into psum[0:mp_, :].
                psum is pre-loaded with ident.T @ x (so psum = s*(x+delta));
                LN stats+apply read the psums; x_sb updated off-chain with
                xscale."""
                st6b = sp.tile([P, 2, 2, 6], F32, tag="lnst6_768")
                mvb = sp.tile([P, 2, 2], F32, tag="lnmv")
                rsb = sp.tile([P, 2], F32, tag="lnrs")
                nmb = sp.tile([P, 2], F32, tag="lnnm")
                vrb = sp.tile([P, 2], F32, tag="lnvr")
                for mt, (mo, mp_) in enumerate(MTS):
                    pss = []
                    for nh in range(2):
                        pb = ps.tile([P, 384], F32, tag="t384", bufs=4,
                                     name=f"rb{tag}_{mt}_{nh}")
                        nc.tensor.matmul(
                            pb[0:mp_, :], ident[0:mp_, 0:mp_],
                            x_sb[0:mp_, mt, 384 * nh:384 * (nh + 1)],
                            start=True, stop=False)
                        emit_nh(pb, mt, mp_, nh)
                        pss.append(pb)
                        nc.vector.bn_stats(st6b[0:mp_, mt, nh], pb[0:mp_, :])
                    nc.vector.bn_aggr(mvb[0:mp_, mt], st6b[0:mp_, mt])
                    nc.vector.tensor_scalar(
                        out=vrb[0:mp_, mt:mt + 1], in0=mvb[0:mp_, mt, 1:2],
                        scalar1=EPS, scalar2=None, op0=ALU.add)
                    nc.vector.reciprocal(rsb[0:mp_, mt:mt + 1],
                                         vrb[0:mp_, mt:mt + 1])
                    nc.scalar.activation(rsb[0:mp_, mt:mt + 1],
                                         rsb[0:mp_, mt:mt + 1], AF.Sqrt)
                    nc.vector.scalar_tensor_tensor(
                        out=nmb[0:mp_, mt:mt + 1], in0=mvb[0:mp_, mt, 0:1],
                        scalar=-1.0, in1=rsb[0:mp_, mt:mt + 1],
                        op0=ALU.mult, op1=ALU.mult)
                    for nh in range(2):
                        nc.scalar.activation(
                            out_t[0:mp_, mt, 384 * nh:384 * (nh + 1)],
                            pss[nh][0:mp_, :], func,
                            bias=nmb[0:mp_, mt:mt + 1],
                            scale=rsb[0:mp_, mt:mt + 1])
                        nc.scalar.activation(
                            x_sb[0:mp_, mt, 384 * nh:384 * (nh + 1)],
                            pss[nh][0:mp_, :], AF.Identity, scale=xscale)

            # ---------- transpose helper (bf16 through PE) ----------
            def transpose_rows(src, out_T, dtiles):
                for d0 in range(0, dtiles, DT):
                    nd = min(DT, dtiles - d0)
                    for mt, (mo, mp_) in enumerate(MTS):
                        pt = ps.tile([P, DT, P], BF16, tag="t384", bufs=4)
                        for dd in range(nd):
                            nc.tensor.transpose(
                                pt[0:P, dd, 0:mp_],
                                src[0:mp_, mt, P * (d0 + dd):P * (d0 + dd + 1)],
                                i16[0:mp_, 0:mp_],
                            )
                        nc.vector.tensor_copy(
                            out_T[:, d0:d0 + nd, mo:mo + mp_],
                            pt[0:P, 0:nd, 0:mp_])

            # ---- remote-DMA all-gather state ----
            # dsem: +2 per arrived send (4 senders incl self = +8/round).
            # lsem: +16 per own completed broadcast (+64/round).
            # csem: +2 per member's consumed-credit (+8/round).
            dsem = nc.alloc_semaphore("rdma_data")
            lsem = nc.alloc_semaphore("rdma_local")
            csem = nc.alloc_semaphore("rdma_credit")
            clsem = nc.alloc_semaphore("rdma_credit_l")
            patches = []

            def issue_bcast(r, xnT8b, trig_prev, cred_prev):
                """Broadcast own xn^T (layout B) to self + 3 XOR peers."""
                if r >= 1:
                    wc = nc.gpsimd.wait_ge(csem, 0)
                    patches.append((wc.ins, 8 * r))
                    _add_dep(wc.ins, trig_prev.ins, True,
                             "credit wait anchored after prev trigger")
                    if cred_prev is not None:
                        _add_dep(wc.ins, cred_prev.ins, True,
                                 "own credit must precede credit wait")
                else:
                    wc = None
                xg = ap.tile([P, 4, DT * RWS], F8, tag="xg",
                             name=f"xg{r}")
                for d in range(4):
                    b = nc.gpsimd.remote_dma_broadcast(
                        xg[:, d, :], xnT8b[:],
                        remote_sem=dsem, local_sem=lsem,
                        rdests=[(0, d)] + [None] * 7)
                    if wc is not None:
                        _add_dep(b.ins, wc.ins, True, "xg credit gate")
                trig = nc.gpsimd.trigger_dma(count=None)
                return trig, xg

            def wait_gather(r, trig):
                """PE-queue wait for all 4 slots of round r to land."""
                w = nc.tensor.wait_ge(dsem, 0)
                patches.append((w.ins, 8 * (r + 1)))
                _add_dep(w.ins, trig.ins, True, "data wait after trigger")
                return w

            def send_credit(r, consumers):
                cr = nc.gpsimd.remote_sem_update_broadcast(
                    csem, clsem,
                    rdests=[(0, 0), (0, 1), (0, 2), (0, 3)] + [None] * 4)
                for ci in consumers:
                    _add_dep(cr.ins, ci.ins, True, "credit after consume")
                # Fire promptly: the boundary trigger runs after the next
                # csem wait, which needs this credit — own trigger avoids
                # the cross-core cycle.
                nc.gpsimd.trigger_dma(count=None)
                return cr

            # ---- L0 xn + broadcast issued first: overlaps the SE ----
            xn_cur = ap.tile([P, 2, D], BF16, tag="xn", name="xn0")
            layernorm(x_stage, D, xn_cur)

            def transpose_rows_b(src, outB, r, trig_prev):
                """Transpose [rows, 768] -> layout B [rb0:(t6,128)|rb1:(t6,64)]
                with the lsem WAR gate for round r's xnT8 buffer."""
                if r >= 2:
                    wl = nc.vector.wait_ge(lsem, 0)
                    patches.append((wl.ins, 64 * (r - 1)))
                    _add_dep(wl.ins, trig_prev.ins, True,
                             "send-WAR wait anchored after prev trigger")
                else:
                    wl = None
                views = [
                    outB[:, 0:DT * P].rearrange("p (t m) -> p t m", t=DT),
                    outB[:, DT * P:DT * RWS].rearrange(
                        "p (t m) -> p t m", t=DT),
                ]
                for mt, (mo, mp_) in enumerate(MTS):
                    pt = ps.tile([P, DT, P], BF16, tag="t384", bufs=4)
                    for dd in range(DT):
                        nc.tensor.transpose(
                            pt[0:P, dd, 0:mp_],
                            src[0:mp_, mt, P * dd:P * (dd + 1)],
                            i16[0:mp_, 0:mp_],
                        )
                    cp = nc.vector.tensor_copy(
                        views[mt][:, :, 0:mp_], pt[0:P, 0:DT, 0:mp_])
                    if wl is not None:
                        _add_dep(cp.ins, wl.ins, True, "xnT8 send-WAR gate")

            xnT8_cur = ap.tile([P, DT * RWS], F8, tag="xnTown", bufs=2,
                               name="xnT0")
            transpose_rows_b(xn_cur, xnT8_cur, 0, None)
            cred_cur = None
            trig_cur, xg_cur = issue_bcast(0, xnT8_cur, None, None)

            # ================= structure encoder (fp8 DR) =================
            siT8 = pp.tile([P, 3, 2 * RWS], F8)
            nc.gpsimd.dma_start(siT8[:], si_in[:])
            wse1 = wp.tile([P, 3, 4, 2, 512], F8, tag="wff1")
            nc.gpsimd.dma_start(wse1[:], wse1_in[:])
            h_sb = ap.tile([P, 2, FF], BF16, tag="hse")
            for mt, (mo, mp_) in enumerate(MTS):
                for nb in range(4):
                    hp = ps.tile([P, 512], F32, tag="t384", bufs=4)
                    for dpi in range(3):
                        nc.tensor.matmul(
                            hp[0:mp_, :], lhsT_blk(siT8, dpi, mt),
                            wse1[:, dpi, nb, :, :],
                            start=(dpi == 0), stop=(dpi == 2), perf_mode=DRM,
                        )
                    nc.vector.tensor_copy(
                        h_sb[0:mp_, mt, 512 * nb:512 * (nb + 1)], hp[0:mp_, :])
            hr = ap.tile([P, 2, FF], BF16, tag="hrse")
            layernorm(h_sb, FF, hr, func=AF.Relu)
            hrT8 = ap.tile([P, FFT, RWS], F8, tag="hrT")
            transpose_rows(hr, hrT8, FFT)
            wse2 = wp.tile([P, 8, 6, 2, P], F8, tag="wff2", bufs=1)
            nc.gpsimd.dma_start(wse2[:], wse2_in[:])
            for m in range(DT):
                sbp = ps.tile([P, RWS], F32, tag="t192o", bufs=2)
                for fp_ in range(8):
                    nc.tensor.matmul(
                        sbp[:], wse2[:, fp_, m, :, :],
                        hrT8[:, 2 * fp_:2 * fp_ + 2, :],
                        start=(fp_ == 0), stop=(fp_ == 7), perf_mode=DRM,
                    )
                nc.scalar.activation(sbT8[:, m, :], sbp[:], AF.Identity,
                                     scale=1.0 / WS)

            # ---- sw^T for layer li: swT bf16 = 2048*sw ----
            def emit_swT(li):
                wsp = wp.tile([P, 3, 6, 2, P], F8, tag="wsp", name=f"wsp{li}")
                nc.gpsimd.dma_start(wsp[:], wsp_in[li])
                swT = ap.tile([P, KT, RWS], BF16, tag="swT", bufs=2,
                              name=f"swT{li}")
                for ktp in range(3):
                    swp = ps.tile([P, 2, RWS], F32, tag="t384", bufs=4)
                    for j2 in range(2):
                        kt = 2 * ktp + j2
                        for dpi in range(3):
                            nc.tensor.matmul(
                                swp[:, j2, :], wsp[:, dpi, kt, :, :],
                                sbT8[:, 2 * dpi:2 * dpi + 2, :],
                                start=(dpi == 0), stop=(dpi == 2),
                                perf_mode=DRM,
                            )
                    nc.scalar.activation(
                        swT[:, 2 * ktp:2 * ktp + 2, :].rearrange(
                            "p a b -> p (a b)"),
                        swp[:].rearrange("p a b -> p (a b)"),
                        AF.Identity, scale=SWS)
                return swT

            swT_cur = emit_swT(0)

            v8 = pp.tile([P, 3, H, 2, P], F8)
            nc.gpsimd.memset(v8[:, :, :, :, HD + 1:P], 0.0)
            nc.gpsimd.memset(v8[:, :, :, :, HD:HD + 1], 1.0)

            # ================= transformer layers =================
            for li in range(NL):
                swT = swT_cur
                wqk = wp.tile([P, 3, 12, 2, P], F8, tag="wqk")
                nc.gpsimd.dma_start(wqk[:], wqk_in[li])
                wv = wp.tile([P, 3, 2, 2, 384], F8, tag="wv")
                nc.gpsimd.dma_start(wv[:], wv_in[li])
                wopt = wp.tile([P, 3, 2, 2, 384], F8, tag="wop")
                nc.gpsimd.dma_start(wopt[:], wop_in[li])
                wff1 = wp.tile([P, DT, FF], BF16, tag="wff1")
                nc.gpsimd.dma_start(
                    wff1[:], wff1_in[li].rearrange("(t p) n -> p t n", p=P))
                wff2 = wp.tile([P, FFT, D], BF16, tag="wff2", bufs=1)
                nc.gpsimd.dma_start(
                    wff2[:], wff2_in[li].rearrange("(t p) n -> p t n", p=P))

                # ---- gather for this layer was issued at the prev boundary --
                xnT8 = xnT8_cur
                trig, xg = trig_cur, xg_cur

                def xgb(s, rb):
                    """Slot s, row-block rb view of the gathered buffer:
                    [P, DT, 128|64] fp8 (t-tiles adjacent -> DR pairs)."""
                    if rb == 0:
                        return xg[:, s, 0:DT * P].rearrange(
                            "p (t m) -> p t m", t=DT)
                    return xg[:, s, DT * P:DT * RWS].rearrange(
                        "p (t m) -> p t m", t=DT)

                xnb = [xnT8[:, 0:DT * P].rearrange("p (t m) -> p t m", t=DT),
                       xnT8[:, DT * P:DT * RWS].rearrange(
                           "p (t m) -> p t m", t=DT)]

                # ---- gather-window work: q^T + next layer's swT ----
                qT8 = ap.tile([P, DT, RWS], F8, tag="qT")
                for fp_ in range(3):
                    qp = ps.tile([P, 2, RWS], F32, tag="t384", bufs=4)
                    for j2 in range(2):
                        f = 2 * fp_ + j2
                        for rb, (ro, rp_) in enumerate(MTS):
                            for dpi in range(3):
                                nc.tensor.matmul(
                                    qp[:, j2, ro:ro + rp_],
                                    wqk[:, dpi, f, :, :],
                                    xnb[rb][:, 2 * dpi:2 * dpi + 2, :],
                                    start=(dpi == 0), stop=(dpi == 2),
                                    perf_mode=DRM,
                                )
                    nc.vector.tensor_copy(
                        qT8[:, 2 * fp_:2 * fp_ + 2, :].rearrange(
                            "p a b -> p (a b)"),
                        qp[:].rearrange("p a b -> p (a b)"))
                if li + 1 < NL:
                    swT_cur = emit_swT(li + 1)

                # ---- wait for all 4 slots, then k^T / v straight from xg ----
                wg = wait_gather(li, trig)
                consumers = []
                kT8 = ap.tile([P, DT, L], F8, tag="kT8")

                def emit_kT(f):
                    kpa = ps.tile([P, 4, P], F32, tag="t512", bufs=2)
                    for s in range(4):
                        for dpi in range(3):
                            m = nc.tensor.matmul(
                                kpa[:, s, :], wqk[:, dpi, 6 + f, :, :],
                                xgb(s, 0)[:, 2 * dpi:2 * dpi + 2, :],
                                start=(dpi == 0), stop=(dpi == 2),
                                perf_mode=DRM,
                            )
                            consumers.append(m)
                    kpb = ps.tile([P, 4, P], F32, tag="t512", bufs=2)
                    for s in range(4):
                        for dpi in range(3):
                            m = nc.tensor.matmul(
                                kpb[:, s, 0:64], wqk[:, dpi, 6 + f, :, :],
                                xgb(s, 1)[:, 2 * dpi:2 * dpi + 2, :],
                                start=(dpi == 0), stop=(dpi == 2),
                                perf_mode=DRM,
                            )
                            consumers.append(m)
                    if f % 2 == 0:
                        nc.scalar.activation(
                            kT8[:, f, 0:4 * P],
                            kpa[:].rearrange("p a b -> p (a b)"), AF.Identity)
                        nc.vector.tensor_copy(
                            kT8[:, f, 4 * P:L].rearrange(
                                "p (a b) -> p a b", a=4),
                            kpb[:, :, 0:64])
                    else:
                        nc.vector.tensor_copy(
                            kT8[:, f, 0:4 * P],
                            kpa[:].rearrange("p a b -> p (a b)"))
                        nc.scalar.activation(
                            kT8[:, f, 4 * P:L].rearrange(
                                "p (a b) -> p a b", a=4),
                            kpb[:, :, 0:64], AF.Identity)

                def emit_v0(s, nh):
                    vp = ps.tile([P, 384], F32, tag="t384", bufs=4)
                    for dpi in range(3):
                        m = nc.tensor.matmul(
                            vp[:], xgb(s, 0)[:, 2 * dpi:2 * dpi + 2, :],
                            wv[:, dpi, nh, :, :],
                            start=(dpi == 0), stop=(dpi == 2),
                            perf_mode=DRM,
                        )
                        consumers.append(m)
                    dst = v8[:, s // 2, 6 * nh:6 * nh + 6, s % 2, 0:HD]
                    src_v = vp[:].rearrange("p (h d) -> p h d", d=HD)
                    if (s + nh) % 2 == 0:
                        nc.vector.tensor_copy(dst, src_v)
                    else:
                        nc.scalar.activation(dst, src_v, AF.Identity)

                # rb1 slot-pairs staged into a contiguous [t, s, r] tile so
                # the DR lhsT has monotonic strides (AP lowering reorders
                # non-monotonic free dims, which breaks the pair dim).
                vstg = ap.tile([P, 2, DT, 2, 64], F8, tag="vstg")
                for sp_ in range(2):
                    reg = xg[:, 2 * sp_:2 * sp_ + 2, DT * P:DT * RWS]
                    regv = reg.rearrange("p s (t r) -> p s t r", t=DT)
                    cps = nc.vector.tensor_copy(
                        vstg[:, sp_].rearrange("p t s r -> p s t r"), regv)
                    consumers.append(cps)

                def emit_v1(sp_, nh):
                    """rb1 of slots (2sp_, 2sp_+1) fused: out partitions =
                    64*s_in_pair + r -> kt 4+sp_."""
                    vp = ps.tile([P, 384], F32, tag="t384", bufs=4)
                    for dpi in range(3):
                        m = nc.tensor.matmul(
                            vp[:],
                            vstg[:, sp_, 2 * dpi:2 * dpi + 2, :, :],
                            wv[:, dpi, nh, :, :],
                            start=(dpi == 0), stop=(dpi == 2),
                            perf_mode=DRM,
                        )
                        consumers.append(m)
                    dst = v8[:, 2, 6 * nh:6 * nh + 6, sp_, 0:HD]
                    src_v = vp[:].rearrange("p (h d) -> p h d", d=HD)
                    if (sp_ + nh) % 2 == 0:
                        nc.vector.tensor_copy(dst, src_v)
                    else:
                        nc.scalar.activation(dst, src_v, AF.Identity)

                for f in range(DT):
                    emit_kT(f)
                for s in range(4):
                    for nh in range(2):
                        emit_v0(s, nh)
                for sp_ in range(2):
                    for nh in range(2):
                        emit_v1(sp_, nh)
                for m in consumers:
                    _add_dep(m.ins, wg.ins, True, "consume after gather wait")
                cred_cur = send_credit(li, consumers)

                # ---- attention ----
                oT8 = ap.tile([P, 3, 384], F8, tag="oT")   # DR-lhsT layout
                for hpair in range(DT):
                    opair = [ps.tile([P, RWS], F32, tag="t192o", bufs=2,
                                     name=f"op{li}_{hpair}_{_h}")
                             for _h in range(2)]
                    pexps_all = []
                    for ktp in range(3):
                        spair = [ps.tile([P, 2, RWS], F32, tag="t384", bufs=4,
                                         name=f"sp{li}_{hpair}_{ktp}_{_h}")
                                 for _h in range(2)]
                        for j in range(2):
                            kt = 2 * ktp + j
                            for hh in range(2):
                                po = 64 * hh
                                nc.tensor.matmul(
                                    spair[hh][:, j, :], i16[:],
                                    swT[:, kt, :],
                                    start=True, stop=False,
                                )
                                nc.tensor.matmul(
                                    spair[hh][:, j, :],
                                    kT8[po:po + HD, hpair, P * kt:P * (kt + 1)],
                                    qT8[po:po + HD, hpair, :],
                                    start=False, stop=True,
                                )
                        pexps = []
                        for hh in range(2):
                            pexp = sp.tile([P, 2, RWS], F8, tag="pexp", bufs=8,
                                           name=f"px{li}_{hpair}_{ktp}_{hh}")
                            pexps.append(pexp)
                            nc.scalar.activation(
                                pexp[:].rearrange("p a b -> p (a b)"),
                                spair[hh][:].rearrange("p a b -> p (a b)"),
                                AF.Exp, scale=EXPS)
                        pexps_all.append(pexps)
                    for ktp in range(3):
                        for hh in range(2):
                            h = 2 * hpair + hh
                            nc.tensor.matmul(
                                opair[hh][:],
                                v8[:, ktp, h, :, :],
                                pexps_all[ktp][hh][:],
                                start=(ktp == 0), stop=(ktp == 2),
                                perf_mode=DRM,
                            )
                    rz = sp.tile([1, 2, RWS], F16, tag="rz", bufs=1)
                    with nc.allow_low_precision(reason="1/z fits fp16"):
                        for hh in range(2):
                            nc.vector.reciprocal(rz[0:1, hh, :],
                                                 opair[hh][HD:HD + 1, :])
                    rbp = ps.tile([P, 384], F32, tag="t384", bufs=4)
                    for hh in range(2):
                        nc.tensor.matmul(rbp[64 * hh:64 * hh + 64, 0:RWS],
                                         emat[0:1, 0:64], rz[0:1, hh, :],
                                         start=True, stop=True)
                    rb = sp.tile([P, RWS], F32, tag="rb")
                    nc.vector.tensor_copy(rb[:], rbp[:, 0:RWS])
                    dpi, jj = divmod(hpair, 2)
                    for hh in range(2):
                        po = 64 * hh
                        nc.vector.tensor_tensor(
                            out=oT8[po:po + HD, dpi, P * jj:P * jj + P],
                            in0=opair[hh][0:HD, 0:P], in1=rb[po:po + HD, 0:P],
                            op=ALU.mult)
                        nc.vector.tensor_tensor(
                            out=oT8[po:po + HD, dpi,
                                    256 + 64 * jj:320 + 64 * jj],
                            in0=opair[hh][0:HD, P:RWS],
                            in1=rb[po:po + HD, P:RWS],
                            op=ALU.mult)

                # ---- output projection + residual + LN2 (psum-fused) ----
                def emit_op(pb, mt, mp_, nh):
                    for dpi in range(3):
                        nc.tensor.matmul(
                            pb[0:mp_, :], lhsT_blk(oT8, dpi, mt),
                            wopt[:, dpi, nh, :, :],
                            start=False, stop=(dpi == 2),
                            perf_mode=DRM,
                        )
                xn2 = ap.tile([P, 2, D], BF16, tag="xn", name=f"xn2_{li}")
                ln_boundary(emit_op, i256f, OPS, xn2, tag=f"o{li}")
                xn2T = ap.tile([P, DT, RWS], BF16, tag="xn2T")
                transpose_rows(xn2, xn2T, DT)
                h1T = ap.tile([P, FFT, RWS], BF16, tag="h1T")
                for fp_ in range(0, FFT, 2):
                    fps_ = ps.tile([P, 2, RWS], F32, tag="t384", bufs=4)
                    for j in range(2):
                        f = fp_ + j
                        for d in range(DT):
                            nc.tensor.matmul(
                                fps_[:, j, :], wff1[:, d, P * f:P * (f + 1)],
                                xn2T[:, d, :],
                                start=(d == 0), stop=(d == DT - 1),
                            )
                    nc.scalar.activation(
                        h1T[:, fp_:fp_ + 2, :].rearrange("p a b -> p (a b)"),
                        fps_[:].rearrange("p a b -> p (a b)"), AF.Relu)
                def emit_ff2(pb, mt, mp_, nh):
                    mo = MTS[mt][0]
                    for f in range(FFT):
                        nc.tensor.matmul(
                            pb[0:mp_, :], h1T[:, f, mo:mo + mp_],
                            wff2[:, f, 384 * nh:384 * (nh + 1)],
                            start=False, stop=(f == FFT - 1),
                        )
                if li + 1 < NL:
                    xn_cur = ap.tile([P, 2, D], BF16, tag="xn",
                                     name=f"xn{li + 1}")
                    ln_boundary(emit_ff2, i1f, 1.0, xn_cur, tag=f"f{li}")
                    xnT8_cur = ap.tile([P, DT * RWS], F8, tag="xnTown",
                                       bufs=2, name=f"xnT{li + 1}")
                    transpose_rows_b(xn_cur, xnT8_cur, li + 1, trig_cur)
                    trig_cur, xg_cur = issue_bcast(li + 1, xnT8_cur, trig_cur,
                                                   cred_cur)
                else:
                    xf = ap.tile([P, 2, D], BF16, tag="xn", name="xf")
                    for mt, (mo, mp_) in enumerate(MTS):
                        for nh, (n0, nw) in enumerate(NT2):
                            f2p = ps.tile([P, 384], F32, tag="t384", bufs=4)
                            nc.tensor.matmul(
                                f2p[0:mp_, :], i1f[0:mp_, 0:mp_],
                                x_sb[0:mp_, mt, n0:n0 + nw],
                                start=True, stop=False)
                            for f in range(FFT):
                                nc.tensor.matmul(
                                    f2p[0:mp_, 0:nw], h1T[:, f, mo:mo + mp_],
                                    wff2[:, f, n0:n0 + nw],
                                    start=False, stop=(f == FFT - 1),
                                )
                            nc.scalar.activation(
                                xf[0:mp_, mt, n0:n0 + nw], f2p[0:mp_, :],
                                AF.Identity)

            # ================= final projection (bf16) =================
            xfT = ap.tile([P, DT, RWS], BF16, tag="xn2T")
            transpose_rows(xf, xfT, DT)
            wout = wp.tile([P, DT, D], BF16, tag="wout", bufs=1)
            nc.gpsimd.dma_start(wout[:], wout_in.rearrange("(t p) n -> p t n", p=P))
            out_sb = pp.tile([P, 2, D], F32)
            for mt, (mo, mp_) in enumerate(MTS):
                for n0, nw in NT2:
                    fop = ps.tile([P, 384], F32, tag="t384", bufs=4)
                    for d in range(DT):
                        nc.tensor.matmul(
                            fop[0:mp_, 0:nw], xfT[:, d, mo:mo + mp_],
                            wout[:, d, n0:n0 + nw],
                            start=(d == 0), stop=(d == DT - 1),
                        )
                    nc.vector.tensor_copy(out_sb[0:mp_, mt, n0:n0 + nw],
                                          fop[0:mp_, 0:nw])
                    nc.sync.dma_start(out_dram[mo:mo + mp_, n0:n0 + nw],
                                      out_sb[0:mp_, mt, n0:n0 + nw])

    for inst, _val in patches:
        assert inst.sync_info.on_wait[0].wait_value == 0
    nc.finalize()
    return nc, patches


def _si_dr(siT):
    """siT [D, RWS] -> [P, 3, 384] DR-lhsT with row blocks (128, 64)."""
    A = siT.reshape(3, 2, P, RWS).transpose(2, 0, 1, 3)   # [p, dp, j, r]
    blk0 = np.ascontiguousarray(A[:, :, :, 0:P]).reshape(P, 3, 256)
    blk1 = np.ascontiguousarray(A[:, :, :, P:RWS]).reshape(P, 3, P)
    return np.concatenate([blk0, blk1], axis=-1)


def kernel(**inputs):
    inp = {k: np.asarray(v, dtype=np.float32) for k, v in inputs.items()}

    qkv_w = inp["qkv_w"].copy()
    qkv_b = inp["qkv_b"].copy()
    for i in range(NL):
        g, b = inp["n1_g"][i], inp["n1_b"][i]
        qkv_b[i] = qkv_b[i] + b @ qkv_w[i]
        qkv_w[i] = g[:, None] * qkv_w[i]
    ff_w1 = inp["ff_w1"].copy()
    ff_b1 = inp["ff_b1"].copy()
    for i in range(NL):
        g, b = inp["n2_g"][i], inp["n2_b"][i]
        ff_b1[i] = ff_b1[i] + b @ ff_w1[i]
        ff_w1[i] = g[:, None] * ff_w1[i]
    sp_b = inp["sp_b"] + inp["se_b2"] @ inp["sp_w"]

    unsupported = []
    for name, arr in [("qkv_b", qkv_b), ("sp_b", sp_b), ("op_b", inp["op_b"]),
                      ("ff_b1", ff_b1), ("ff_b2", inp["ff_b2"]),
                      ("se_b1", inp["se_b1"]), ("out_b", inp["out_b"])]:
        if np.abs(arr).max() > 0:
            unsupported.append(name)
    if (inp["se_g"] != 1).any() or (inp["se_be"] != 0).any():
        unsupported.append("se_affine")
    assert not unsupported, f"nonzero biases not yet supported: {unsupported}"

    wqk = np.stack([_dr(qkv_w[i][:, 0:2 * D], P) for i in range(NL)])
    wv = np.stack([_dr(qkv_w[i][:, 2 * D:3 * D], 384) for i in range(NL)])
    wop = np.stack([_dr(inp["op_w"][i], 384) for i in range(NL)])
    wse1 = _dr(inp["se_w1"], 512)
    wse2 = _dr(inp["se_w2"], P)

    # Per-member sp_w column permutation: the remote-dma gather orders keys
    # as [s0rb0(128) s1rb0 s2rb0 s3rb0 | s0rb1(64) s1rb1 s2rb1 s3rb1] where
    # slot s holds core (c ^ s)'s rows. Attention is key-permutation
    # invariant, so only the structure bias columns must follow.
    wsp_m = []
    for m in range(4):
        perm = np.empty(L, dtype=np.int64)
        k = np.arange(512)
        perm[0:512] = RWS * ((k // P) ^ m) + (k % P)
        k = np.arange(256)
        perm[512:768] = RWS * ((k // 64) ^ m) + P + (k % 64)
        perm_inv = perm  # perm maps key -> global token; columns = keys
        wsp_m.append(np.stack(
            [_dr(inp["sp_w"][i][:, perm_inv], P) for i in range(NL)]))

    emat_np = np.zeros((2, P), dtype=np.float16)
    emat_np[0, 0:HD] = 1.0
    emat_np[1, HD:2 * HD] = 1.0
    if "nc" not in _CACHE:
        _CACHE["nc"], _CACHE["patches"] = build_nc()
    nc = _CACHE["nc"]

    in_maps = []
    for c in range(8):
        b, j = divmod(c, 4)
        rows = slice(RWS * j, RWS * (j + 1))
        in_maps.append({
            "x_rows": np.ascontiguousarray(inp["x"][b, rows]),
            "siT8": _f8(_si_dr(np.ascontiguousarray(
                inp["structure_info"][b, rows].T))),
            "wse1": _f8(wse1, WS), "wse2": _f8(wse2, WS),
            "wqk": _f8(wqk, WS), "wv": _f8(wv, WS),
            "wsp": _f8(wsp_m[j], WS), "wop": _f8(wop, WS),
            "wff1": _bf(ff_w1), "wff2": _bf(inp["ff_w2"]),
            "wout": _bf(inp["out_w"]), "emat": emat_np,
        })

    # Real cross-core wait thresholds for the HW run only; reverted after so
    # single-core sims (TimelineSim) of this module stay deadlock-free.
    import os as _os
    if _os.environ.get("NO_RDMA_WAITS", "0") != "1":
        for inst, val in _CACHE["patches"]:
            inst.sync_info.on_wait[0].wait_value = val
    try:
        res = run_bass_kernel_spmd(nc, in_maps, core_ids=list(range(8)),
                                   **_CACHE.get("run_kwargs", {}))
    finally:
        for inst, _val in _CACHE["patches"]:
            inst.sync_info.on_wait[0].wait_value = 0
    _CACHE["last_result"] = res
    out = np.zeros((B, L, D), dtype=np.float32)
    for c in range(8):
        b, j = divmod(c, 4)
        out[b, RWS * j:RWS * (j + 1)] = res.results[c]["out_rows"]
    return out


if __name__ == "__main__":
    import reference as R
    import os
    os.environ["JAX_PLATFORMS"] = "cpu"
    inputs = {k: np.asarray(v) for k, v in R.setup_inputs().items()}
    got = kernel(**inputs)
    import jax.numpy as jnp
    want = np.asarray(R.reference(**{k: jnp.asarray(v) for k, v in inputs.items()}))
    err = np.abs(got - want).max() / np.abs(want).max()
    print("rel err:", err)

